# revision 8
# baseline (speedup 1.0000x reference)
# Fused conv3x3(same) + bias + tanh + x2 + stride-4 subsample, data-parallel
# over 8 NeuronCores.
#
# Math: out[b,oc,y,x] = 2*tanh(sum_{ic,ky,kx} w[oc,ic,ky,kx]*x[b,ic,4y+ky-1,4x+kx-1] + bias[oc])
# computed in fp16 like the reference. Since the spatial stride (4) exceeds the
# kernel size (3), every output pixel reads a disjoint 3x3x8 input patch, so the
# conv lowers exactly to a [72 -> 64] GEMM over 64*64 pixels per image. The host
# does the im2col rearrangement (pure data movement, fp16 cast is identical to
# the reference's .astype(float16)); each core runs the GEMM + bias + tanh for
# 4 of the 32 images. The trailing *2 and fp32 cast are exact in either order,
# so they are applied on the host after the fp16 tanh.
#
# Device kernel is hand-scheduled raw bacc (no Tile framework: avoids its
# multi-microsecond preamble/tail barriers). Per image, the 8 N=512 matmul
# chunks are packed two-deep in PSUM partitions (chunk 2q+t -> partitions
# t*64:(t+1)*64 of bank q) so one 128-partition ACT evaluates tanh for a whole
# image and the output DMA engages all SBUF ports. Output DRAM layout is
# [B, 2, 64, 2048] (t = chunk parity); the host interleaves it back. The input
# SBUF tile keeps a 64-byte gap between image halves so DMA descriptors stay at
# 4 KiB (measured ~18 GB/s per SDMA engine vs ~14 GB/s at 8 KiB).
import sys

import numpy as np

try:
    import concourse.bass as bass  # noqa: F401
except ImportError:
    sys.path.insert(0, "/opt/trn_rl_repo")

import concourse.bass as bass  # noqa: F401
import concourse.bacc as bacc
import concourse.mybir as mybir
from concourse.bass_utils import run_bass_kernel_spmd

N_CORES = 8
B_FULL = 32
B_CORE = B_FULL // N_CORES  # 4 images per core
C_IN = 8
KH = KW = 3
K = C_IN * KH * KW  # 72 contraction
OC = 64
OH = OW = 64
NPIX = OH * OW  # 4096
HALF = NPIX // 2  # 2048
F16 = mybir.dt.float16
F32 = mybir.dt.float32

_PROGRAM = None


def build_program():
    nc = bacc.Bacc("TRN2")
    xp = nc.dram_tensor("xp", [B_CORE, K, 2, HALF], F16, kind="ExternalInput")
    w = nc.dram_tensor("w", [K, OC], F16, kind="ExternalInput")
    bias = nc.dram_tensor("bias", [2 * OC, 1], F16, kind="ExternalInput")
    y = nc.dram_tensor("y", [B_CORE, 2, OC, HALF], F16, kind="ExternalOutput")

    with (
        nc.sbuf_tensor([K, OC], F16) as w_tile,
        nc.sbuf_tensor([2 * OC, 1], F16) as b_tile,
        # gap of 32 elems between halves keeps per-descriptor runs at 4KiB
        nc.sbuf_tensor([K, 2, HALF + 32], F16) as x0,
        nc.sbuf_tensor([K, 2, HALF + 32], F16) as x1,
        nc.sbuf_tensor([2 * OC, HALF], F16) as a0,
        nc.sbuf_tensor([2 * OC, HALF], F16) as a1,
        nc.psum_tensor([2 * OC, HALF], F32) as p0,
        nc.psum_tensor([2 * OC, HALF], F32) as p1,
        # Per-buffer-slot DMA semaphores: concurrent DMAs complete out of
        # order, so a single counting sem can't tell which transfer landed.
        # Same-slot DMAs are serialized by the consumer chain, so per-slot
        # counts are race-free.
        nc.semaphore("s_w") as s_w,
        nc.semaphore("s_x0") as s_x0,
        nc.semaphore("s_x1") as s_x1,
        nc.semaphore("s_mm") as s_mm,
        nc.semaphore("s_act") as s_act,
        nc.semaphore("s_y0") as s_y0,
        nc.semaphore("s_y1") as s_y1,
        nc.Block() as block,
    ):
        xb = [x0, x1]
        ab = [a0, a1]
        pb = [p0, p1]
        sx = [s_x0, s_x1]
        sy = [s_y0, s_y1]

        @block.sync
        def _(sync):
            sync.dma_start(out=w_tile[:], in_=w[:]).then_inc(s_w, 16)
            sync.dma_start(out=b_tile[:], in_=bias[:]).then_inc(s_w, 16)
            # interleave loads and stores to avoid FIFO head-of-line blocking:
            # in0, in1, in2|out0, in3|out1, out2, out3
            for b in range(B_CORE + 2):
                if b < B_CORE:
                    if b >= 2:
                        # x buffer reused; wait until matmuls of image b-2 done
                        sync.wait_ge(s_mm, b - 1)
                    sync.dma_start(
                        out=xb[b % 2][:, :, :HALF], in_=xp[b]
                    ).then_inc(sx[b % 2], 16)
                if b >= 2:
                    ob = b - 2
                    sync.wait_ge(s_act, ob + 1)
                    sync.dma_start(
                        out=y[ob, 0], in_=ab[ob % 2][:OC, :]
                    ).then_inc(sy[ob % 2], 16)
                    sync.dma_start(
                        out=y[ob, 1], in_=ab[ob % 2][OC:, :]
                    ).then_inc(sy[ob % 2], 16)
            sync.wait_ge(s_y0, 32 * (B_CORE // 2))
            sync.wait_ge(s_y1, 32 * (B_CORE // 2))

        @block.tensor
        def _(tensor):
            for b in range(B_CORE):
                if b == 0:
                    tensor.wait_ge(s_w, 32)
                tensor.wait_ge(sx[b % 2], 16 * (b // 2 + 1))
                if b >= 2:
                    # psum buffer reused; wait until ACT of image b-2 read it
                    tensor.wait_ge(s_act, b - 1)
                last = None
                for t in range(2):
                    for q in range(4):
                        j = 2 * q + t
                        last = nc.tensor.matmul(
                            pb[b % 2][t * OC : (t + 1) * OC, q * 512 : (q + 1) * 512],
                            w_tile[:],
                            xb[b % 2][:, j // 4, (j % 4) * 512 : (j % 4 + 1) * 512],
                            start=True,
                            stop=True,
                        )
                last.then_inc(s_mm, 1)

        @block.scalar
        def _(scalar):
            for b in range(B_CORE):
                scalar.wait_ge(s_mm, b + 1)
                if b >= 2:
                    # act buffer reused; wait until out-DMAs of image b-2 done
                    scalar.wait_ge(sy[b % 2], 32 * (b // 2))
                nc.scalar.activation(
                    ab[b % 2][:],
                    pb[b % 2][:],
                    mybir.ActivationFunctionType.Tanh,
                    bias=b_tile[:],
                ).then_inc(s_act, 1)

    nc.finalize()
    return nc


def _get_program():
    global _PROGRAM
    if _PROGRAM is None:
        _PROGRAM = build_program()
    return _PROGRAM


def _im2col(x: np.ndarray) -> np.ndarray:
    """[B,8,256,256] fp32 -> [B,72,4096] fp16 patches, p=(ky*3+kx)*8+ic."""
    B, C, H, W = x.shape
    xh = x.astype(np.float16)
    xpad = np.zeros((B, C, H + 2, W + 2), np.float16)
    xpad[:, :, 1 : H + 1, 1 : W + 1] = xh
    s = xpad.strides
    # windows[b,c,ky,kx,y,x] = xpad[b,c,4y+ky,4x+kx] = x[b,c,4y+ky-1,4x+kx-1]
    win = np.lib.stride_tricks.as_strided(
        xpad,
        shape=(B, C, KH, KW, OH, OW),
        strides=(s[0], s[1], s[2], s[3], 4 * s[2], 4 * s[3]),
    )
    return win.transpose(0, 2, 3, 1, 4, 5).reshape(B, K, NPIX)


def run_sharded(x, weight, bias, **spmd_kwargs):
    """Returns (output, BassKernelResults). spmd_kwargs e.g. trace=True."""
    patches = _im2col(x)  # [32, 72, 4096] f16, contiguous
    w_mat = np.ascontiguousarray(
        weight.transpose(2, 3, 1, 0).reshape(K, OC).astype(np.float16)
    )
    b_half = bias.astype(np.float16).reshape(OC, 1)
    b_mat = np.ascontiguousarray(np.concatenate([b_half, b_half], axis=0))

    in_maps = [
        {
            "xp": patches[c * B_CORE : (c + 1) * B_CORE].reshape(B_CORE, K, 2, HALF),
            "w": w_mat,
            "bias": b_mat,
        }
        for c in range(N_CORES)
    ]
    nc = _get_program()
    res = run_bass_kernel_spmd(nc, in_maps, list(range(N_CORES)), **spmd_kwargs)
    # y core shard: [4, 2, 64, 2048]; pixel chunk j = 2q+t of 512
    y16 = np.concatenate([r["y"] for r in res.results], axis=0)  # [32,2,64,2048]
    y16 = (
        y16.reshape(B_FULL, 2, OC, 4, 512)
        .transpose(0, 2, 3, 1, 4)
        .reshape(B_FULL, OC, NPIX)
    )
    # 2*tanh in fp16 then cast to fp32 == cast then *2 (exact: *2 is an
    # exponent bump, in-range for |tanh|<=1)
    out = y16.astype(np.float32).reshape(B_FULL, OC, OH, OW) * np.float32(2.0)
    return out, res


def kernel(x: np.ndarray, weight: np.ndarray, bias: np.ndarray) -> np.ndarray:
    return run_sharded(x, weight, bias)[0]


# revision 11
# speedup vs baseline: 1.2257x; 1.2257x over previous
# Fused conv3x3(same) + bias + tanh + x2 + stride-4 subsample, data-parallel
# over 8 NeuronCores.
#
# Math: out[b,oc,y,x] = 2*tanh(sum_{ic,ky,kx} w[oc,ic,ky,kx]*x[b,ic,4y+ky-1,4x+kx-1] + bias[oc])
# computed in fp16 like the reference. Since the spatial stride (4) exceeds the
# kernel size (3), every output pixel reads a disjoint 3x3x8 input patch, so the
# conv lowers exactly to a [72 -> 64] GEMM over 64*64 pixels per image. The host
# does the im2col rearrangement (pure data movement, fp16 cast is identical to
# the reference's .astype(float16)); each core runs the GEMM + bias + tanh for
# 4 of the 32 images. The trailing *2 and fp32 cast are exact in either order,
# so they are applied on the host after the fp16 tanh.
#
# Device kernel is hand-scheduled raw bacc (no Tile framework: avoids its
# multi-microsecond preamble/tail barriers). Per image, the 8 N=512 matmul
# chunks are packed two-deep in PSUM partitions (chunk 2q+t -> partitions
# t*64:(t+1)*64 of bank q) so one 128-partition ACT evaluates tanh for a whole
# image and the output DMA engages all SBUF ports. Output DRAM layout is
# [B, 2, 64, 2048] (t = chunk parity); the host interleaves it back. The input
# SBUF tile keeps a 64-byte gap between image halves so DMA descriptors stay at
# 4 KiB (measured ~18 GB/s per SDMA engine vs ~14 GB/s at 8 KiB).
import sys

import numpy as np

try:
    import concourse.bass as bass  # noqa: F401
except ImportError:
    sys.path.insert(0, "/opt/trn_rl_repo")

import concourse.bass as bass  # noqa: F401
import concourse.bacc as bacc
import concourse.mybir as mybir
from concourse.bass_utils import run_bass_kernel_spmd

N_CORES = 8
B_FULL = 32
B_CORE = B_FULL // N_CORES  # 4 images per core
C_IN = 8
KH = KW = 3
K = C_IN * KH * KW  # 72 contraction
OC = 64
OH = OW = 64
NPIX = OH * OW  # 4096
HALF = NPIX // 2  # 2048
F16 = mybir.dt.float16
F32 = mybir.dt.float32

_PROGRAM = None


def build_program():
    nc = bacc.Bacc("TRN2")
    xp = nc.dram_tensor("xp", [B_CORE, K, 2, HALF], F16, kind="ExternalInput")
    w = nc.dram_tensor("w", [K, OC], F16, kind="ExternalInput")
    bias = nc.dram_tensor("bias", [2 * OC, 1], F16, kind="ExternalInput")
    y = nc.dram_tensor("y", [B_CORE, 2, OC, HALF], F16, kind="ExternalOutput")

    with (
        nc.sbuf_tensor([K, OC], F16) as w_tile,
        nc.sbuf_tensor([2 * OC, 1], F16) as b_tile,
        # gap of 32 elems between halves keeps per-descriptor runs at 4KiB;
        # one x and one act buffer per image -> no buffer-reuse waits
        nc.sbuf_tensor([K, B_CORE, 2, HALF + 32], F16) as x_bufs,
        nc.sbuf_tensor([2 * OC, B_CORE, HALF], F16) as a_bufs,
        nc.psum_tensor([2 * OC, HALF], F32) as p0,
        nc.psum_tensor([2 * OC, HALF], F32) as p1,
        # Per-image input semaphores: concurrent DMAs complete out of order,
        # so a single counting sem can't tell which transfer landed. s_y only
        # gates the final all-done wait, where order doesn't matter.
        nc.semaphore("s_w") as s_w,
        nc.semaphore("s_x0") as s_x0,
        nc.semaphore("s_x1") as s_x1,
        nc.semaphore("s_x2") as s_x2,
        nc.semaphore("s_x3") as s_x3,
        nc.semaphore("s_mm") as s_mm,
        nc.semaphore("s_act") as s_act,
        nc.semaphore("s_y") as s_y,
        nc.Block() as block,
    ):
        pb = [p0, p1]
        sx = [s_x0, s_x1, s_x2, s_x3]

        @block.sync
        def _(sync):
            # image 0 first: it heads the critical path; w/bias are tiny
            sync.dma_start(out=x_bufs[:, 0, :, :HALF], in_=xp[0]).then_inc(sx[0], 16)
            sync.dma_start(out=w_tile[:], in_=w[:]).then_inc(s_w, 16)
            sync.dma_start(out=b_tile[:], in_=bias[:]).then_inc(s_w, 16)
            for b in range(1, B_CORE):
                sync.dma_start(
                    out=x_bufs[:, b, :, :HALF], in_=xp[b]
                ).then_inc(sx[b], 16)

        @block.tensor
        def _(tensor):
            for b in range(B_CORE):
                if b == 0:
                    tensor.wait_ge(s_w, 32)
                tensor.wait_ge(sx[b], 16)
                if b >= 2:
                    # psum buffer reused; wait until ACT of image b-2 read it
                    tensor.wait_ge(s_act, b - 1)
                last = None
                for t in range(2):
                    for q in range(4):
                        j = 2 * q + t
                        last = nc.tensor.matmul(
                            pb[b % 2][t * OC : (t + 1) * OC, q * 512 : (q + 1) * 512],
                            w_tile[:],
                            x_bufs[:, b, j // 4, (j % 4) * 512 : (j % 4 + 1) * 512],
                            start=True,
                            stop=True,
                        )
                last.then_inc(s_mm, 1)

        @block.scalar
        def _(scalar):
            for b in range(B_CORE):
                scalar.wait_ge(s_mm, b + 1)
                nc.scalar.activation(
                    a_bufs[:, b],
                    pb[b % 2][:],
                    mybir.ActivationFunctionType.Tanh,
                    bias=b_tile[:],
                ).then_inc(s_act, 1)
                # one [128, 2048] store per image; y[b] flattens to exactly
                # the act-tile layout. Issued here so it follows the ACT
                # without a cross-engine hop. The explicit wait is required:
                # the engine pipeline lets the DMA trigger race the ACT write.
                scalar.wait_ge(s_act, b + 1)
                scalar.dma_start(
                    out=y[b].rearrange("t o c -> (t o) c"), in_=a_bufs[:, b]
                ).then_inc(s_y, 16)
            scalar.wait_ge(s_y, 16 * B_CORE)

    nc.finalize()
    return nc


def _get_program():
    global _PROGRAM
    if _PROGRAM is None:
        _PROGRAM = build_program()
    return _PROGRAM


def _im2col(x: np.ndarray) -> np.ndarray:
    """[B,8,256,256] fp32 -> [B,72,4096] fp16 patches, p=(ky*3+kx)*8+ic."""
    B, C, H, W = x.shape
    xh = x.astype(np.float16)
    xpad = np.zeros((B, C, H + 2, W + 2), np.float16)
    xpad[:, :, 1 : H + 1, 1 : W + 1] = xh
    s = xpad.strides
    # windows[b,c,ky,kx,y,x] = xpad[b,c,4y+ky,4x+kx] = x[b,c,4y+ky-1,4x+kx-1]
    win = np.lib.stride_tricks.as_strided(
        xpad,
        shape=(B, C, KH, KW, OH, OW),
        strides=(s[0], s[1], s[2], s[3], 4 * s[2], 4 * s[3]),
    )
    return win.transpose(0, 2, 3, 1, 4, 5).reshape(B, K, NPIX)


def run_sharded(x, weight, bias, **spmd_kwargs):
    """Returns (output, BassKernelResults). spmd_kwargs e.g. trace=True."""
    patches = _im2col(x)  # [32, 72, 4096] f16, contiguous
    w_mat = np.ascontiguousarray(
        weight.transpose(2, 3, 1, 0).reshape(K, OC).astype(np.float16)
    )
    b_half = bias.astype(np.float16).reshape(OC, 1)
    b_mat = np.ascontiguousarray(np.concatenate([b_half, b_half], axis=0))

    in_maps = [
        {
            "xp": patches[c * B_CORE : (c + 1) * B_CORE].reshape(B_CORE, K, 2, HALF),
            "w": w_mat,
            "bias": b_mat,
        }
        for c in range(N_CORES)
    ]
    nc = _get_program()
    res = run_bass_kernel_spmd(nc, in_maps, list(range(N_CORES)), **spmd_kwargs)
    # y core shard: [4, 2, 64, 2048]; pixel chunk j = 2q+t of 512
    y16 = np.concatenate([r["y"] for r in res.results], axis=0)  # [32,2,64,2048]
    y16 = (
        y16.reshape(B_FULL, 2, OC, 4, 512)
        .transpose(0, 2, 3, 1, 4)
        .reshape(B_FULL, OC, NPIX)
    )
    # 2*tanh in fp16 then cast to fp32 == cast then *2 (exact: *2 is an
    # exponent bump, in-range for |tanh|<=1)
    out = y16.astype(np.float32).reshape(B_FULL, OC, OH, OW) * np.float32(2.0)
    return out, res


def kernel(x: np.ndarray, weight: np.ndarray, bias: np.ndarray) -> np.ndarray:
    return run_sharded(x, weight, bias)[0]


# revision 21
# speedup vs baseline: 1.4032x; 1.1448x over previous
# Fused conv3x3(same) + bias + tanh + x2 + stride-4 subsample, data-parallel
# over 8 NeuronCores.
#
# Math: out[b,oc,y,x] = 2*tanh(sum_{ic,ky,kx} w[oc,ic,ky,kx]*x[b,ic,4y+ky-1,4x+kx-1] + bias[oc])
# computed in fp16 like the reference. Since the spatial stride (4) exceeds the
# kernel size (3), every output pixel reads a disjoint 3x3x8 input patch, so the
# conv lowers exactly to a [72 -> 64] GEMM over 64*64 pixels per image. The host
# does the im2col rearrangement (pure data movement, fp16 cast is identical to
# the reference's .astype(float16)); each core runs the GEMM + bias + tanh for
# 4 of the 32 images. The trailing *2 and fp32 cast are exact in either order,
# so they are applied on the host after the fp16 tanh.
#
# Device kernel is hand-scheduled raw bacc (no Tile framework: avoids its
# multi-microsecond preamble/tail barriers). The pipeline works in half-images
# (2048 pixels): 4 N=512 matmuls packed two-deep in PSUM partitions (chunk
# 2q+t -> partitions t*64:(t+1)*64 of bank q) so one 128-partition ACT computes
# tanh per half and the output DMA engages all SBUF ports. Output DRAM layout
# is [B, 2, 64, 2048] (t = chunk parity); the host interleaves it back.
#
# The contraction is zero-padded 72 -> 80 rows: an 80-partition DMA spreads
# over all 16 SDMA engines (a 72-partition one only gets 12), which is worth
# more than the 11% extra bytes — the kernel is input-DMA-stream-bound.
# Per-descriptor runs are kept at 4 KiB (~17 GB/s per engine vs ~14 at 8 KiB).
import sys

import numpy as np

try:
    import concourse.bass as bass  # noqa: F401
except ImportError:
    sys.path.insert(0, "/opt/trn_rl_repo")

import concourse.bass as bass  # noqa: F401
import concourse.bacc as bacc
import concourse.mybir as mybir
from concourse.bass_utils import run_bass_kernel_spmd

N_CORES = 8
B_FULL = 32
B_CORE = B_FULL // N_CORES  # 4 images per core
C_IN = 8
KH = KW = 3
K = C_IN * KH * KW  # 72 contraction
KP = 80  # zero-padded contraction (16-SDMA-engine alignment)
OC = 64
OH = OW = 64
NPIX = OH * OW  # 4096
HALF = NPIX // 2  # 2048
NH = 2 * B_CORE  # 8 half-image pipeline stages
F16 = mybir.dt.float16
F32 = mybir.dt.float32

_PROGRAM = None


def build_program():
    from contextlib import ExitStack

    nc = bacc.Bacc("TRN2")
    xp = nc.dram_tensor("xp", [B_CORE, KP, 2, HALF], F16, kind="ExternalInput")
    w = nc.dram_tensor("w", [KP, OC], F16, kind="ExternalInput")
    y = nc.dram_tensor("y", [NH, 2 * OC, HALF // 2], F16, kind="ExternalOutput")

    with ExitStack() as stack:
        w_tile = stack.enter_context(nc.sbuf_tensor([KP, OC], F16))
        # one buffer per half-image stage -> no buffer-reuse waits; each DMA
        # writes one contiguous 4KiB run per partition
        x_bufs = stack.enter_context(nc.sbuf_tensor([KP, NH, HALF], F16))
        a_bufs = stack.enter_context(nc.sbuf_tensor([2 * OC, NH, HALF // 2], F16))
        warm = stack.enter_context(nc.sbuf_tensor([2 * OC, 2 * OC], F16))
        # 8 banks of [128, 512]; stage i accumulates into banks 2i%8, 2i%8+1
        ps = stack.enter_context(nc.psum_tensor([2 * OC, 8, 512], F32))
        # Per-stage input semaphores: concurrent DMAs complete out of order,
        # so one counting sem can't tell which transfer landed. s_y only
        # gates the final all-done wait, where order doesn't matter.
        sx = [stack.enter_context(nc.semaphore(f"s_x{i}")) for i in range(NH)]
        s_w = stack.enter_context(nc.semaphore("s_w"))
        s_warm = stack.enter_context(nc.semaphore("s_warm"))
        s_mm = stack.enter_context(nc.semaphore("s_mm"))
        s_act = stack.enter_context(nc.semaphore("s_act"))
        s_y = stack.enter_context(nc.semaphore("s_y"))
        block = stack.enter_context(nc.Block())

        @block.gpsimd
        def _(gpsimd):
            gpsimd.memset(warm[:], 0.0).then_inc(s_warm, 1)

        @block.sync
        def _(sync):
            # first half-image heads the critical path; w is tiny. The bias
            # rides in w row K (patch row K is constant 1.0), so there is no
            # separate bias operand anywhere.
            sync.dma_start(out=x_bufs[:, 0, :], in_=xp[0][:, 0, :]).then_inc(sx[0], 16)
            sync.dma_start(out=w_tile[:], in_=w[:]).then_inc(s_w, 16)
            for i in range(1, NH):
                sync.dma_start(
                    out=x_bufs[:, i, :], in_=xp[i // 2][:, i % 2, :]
                ).then_inc(sx[i], 16)
            # output stores, paced by the ACT chain; the scalar queue must
            # not carry them (a trigger costs ~0.6us and would serialize
            # with the 1.1us ACTs)
            for i in range(NH - 1):
                sync.wait_ge(s_act, i + 1)
                sync.dma_start(out=y[i], in_=a_bufs[:, i]).then_inc(s_y, 16)
            for q in range(2):
                sync.wait_ge(s_act, NH + q)
                sync.dma_start(
                    out=y[NH - 1][:, 512 * q : 512 * (q + 1)],
                    in_=a_bufs[:, NH - 1, 512 * q : 512 * (q + 1)],
                ).then_inc(s_y, 16)
            sync.wait_ge(s_y, 16 * (NH + 1))

        @block.tensor
        def _(tensor):
            # keep the PE busy while inputs stream in so the HAM clock gate
            # opens (cold MMs run at 1.2GHz, warm at 2.4GHz); results land in
            # bank 7 which is overwritten by stage 3 later (start=True)
            tensor.wait_ge(s_warm, 1)
            for _ in range(50):
                nc.tensor.matmul(
                    ps[:OC, 7, :128],
                    warm[:, :OC],
                    warm[:],
                    start=True,
                    stop=True,
                )
            for i in range(NH):
                if i == 0:
                    tensor.wait_ge(s_w, 16)
                if i >= 4:
                    # psum bank pair reused; wait until ACT of stage i-4 read
                    # it. Taken BEFORE the input wait so the fillers below may
                    # touch this stage's banks.
                    tensor.wait_ge(s_act, i - 3)
                    # fillers: keep the PE busy while waiting for this
                    # stage's input so the HAM clock gate stays open (late
                    # stages otherwise re-throttle to 1.2GHz). They write
                    # this stage's own first bank, which the real start=True
                    # matmuls overwrite.
                    for _ in range(3):
                        nc.tensor.matmul(
                            ps[:OC, (2 * i) % 8, :128],
                            warm[:, :OC],
                            warm[:],
                            start=True,
                            stop=True,
                        )
                tensor.wait_ge(sx[i], 16)
                last = None
                for q in range(2):
                    for t in range(2):
                        c = 2 * q + t  # chunk within this half-image
                        last = nc.tensor.matmul(
                            ps[t * OC : (t + 1) * OC, (2 * i + q) % 8, :],
                            w_tile[:],
                            x_bufs[:, i, c * 512 : (c + 1) * 512],
                            start=True,
                            stop=True,
                        )
                    if q == 1 or i == NH - 1:
                        # the last stage signals per 2-MM burst so the ACT
                        # and store chain drains in quarters
                        last.then_inc(s_mm, 1)

        @block.scalar
        def _(scalar):
            for i in range(NH - 1):
                scalar.wait_ge(s_mm, i + 1)
                bk = (2 * i) % 8
                nc.scalar.activation(
                    a_bufs[:, i],
                    ps[:, bk : bk + 2, :].rearrange("p b c -> p (b c)"),
                    mybir.ActivationFunctionType.Tanh,
                ).then_inc(s_act, 1)
            for q in range(2):
                scalar.wait_ge(s_mm, NH + q)
                nc.scalar.activation(
                    a_bufs[:, NH - 1, 512 * q : 512 * (q + 1)],
                    ps[:, (2 * (NH - 1) + q) % 8, :],
                    mybir.ActivationFunctionType.Tanh,
                ).then_inc(s_act, 1)

    nc.finalize()
    return nc
